# revision 27
# baseline (speedup 1.0000x reference)
"""Trainium2 Bass kernel for nn_AttentionModule (B=2, S=2048, D=1024, H=16).

Sharding: 8 cores = 2 batches x 4 query blocks. Core c handles batch b=c//4,
query rows q0=(c%4)*SQ .. +SQ (SQ=512). Each core computes K and V for the
FULL batch locally (replicated within the 4-core batch group - cheaper and
far more predictable than exchanging K/V shards through the 4-rank AllGather,
whose latency measures ~140us on this part), plus Q for its own rows, then
runs attention for its query block and the output projection. Host-side
assembly is a pure concat.

Device math (all matmuls bf16 inputs, fp32 accumulation):
  K_T [D, S]     = Wk^T-tiles @ x_T    (+bias+pe via DVE per-partition add)
  V   [S, H, 65] = x_T-tiles @ Wv^T    (+bias; col 64 holds ones)
  Q_T [D, SQ]    = Wq^T-tiles @ xq_T   (own rows)
  S_T[k,q] = K_T^t @ Q_T per head (row-packed pairs, d=64)
  P = exp(S/8) on ScalarE (scale fused), masked by 0/1 bf16 multiply on DVE
  attn_T[dv,q] = V_aug^t @ P per head; row 64 accumulates the softmax sums
  attn = attn_T * recip(sums)  -> fc: out[q,e] = attn_T-tiles @ fcw^T-tiles

The attention loop is software-pipelined by one kt so the PE FIFO always has
the next QK pair ahead of the current PV, keeping the ScalarE exp stream (the
pacing engine) dense.
"""

import math

import ml_dtypes
import numpy as np

import concourse.bass as bass
import concourse.mybir as mybir
import concourse.tile as tile
from concourse import bacc
from concourse.bass_utils import run_bass_kernel_spmd

B = 2
D = 1024
H = 16
HD = 64
P = 128
NCORES = 8
F32 = mybir.dt.float32
BF16 = mybir.dt.bfloat16


def build_nc(S=2048):
    SQ = S // 4          # query rows per core
    DT = D // P          # 8 d-model tiles
    KT = S // P          # key tiles
    QC = SQ // P         # query chunks for fc
    SC = max(S // 512, 2)  # 512-wide chunks in projection psum (>=2 for V)
    QSC = SQ // 512 if SQ >= 512 else 1
    QW = min(SQ, 512)    # q projection chunk width
    GROUPS = H // 2      # head pairs

    nc = bacc.Bacc(None)

    xT = nc.dram_tensor("xT", [D, S], BF16, kind="ExternalInput")
    xqT = nc.dram_tensor("xqT", [D, SQ], BF16, kind="ExternalInput")
    wqT = nc.dram_tensor("wqT", [D, D], BF16, kind="ExternalInput")
    wkT = nc.dram_tensor("wkT", [D, D], BF16, kind="ExternalInput")
    wvT = nc.dram_tensor("wvT", [D, D], BF16, kind="ExternalInput")
    fcwT = nc.dram_tensor("fcwT", [D, D], BF16, kind="ExternalInput")
    bq = nc.dram_tensor("bq", [D], F32, kind="ExternalInput")
    bk = nc.dram_tensor("bk", [D], F32, kind="ExternalInput")
    bv = nc.dram_tensor("bv", [D], F32, kind="ExternalInput")
    fcb = nc.dram_tensor("fcb", [D], F32, kind="ExternalInput")
    m01T = nc.dram_tensor("m01T", [S, SQ], BF16, kind="ExternalInput")
    out = nc.dram_tensor("out", [SQ, D], F32, kind="ExternalOutput")

    with tile.TileContext(nc) as tc:
        with tc.tile_pool(name="sing", bufs=1) as sing:
            # ---- resident SBUF tensors ----
            xT_sb = sing.tile([P, DT, S], BF16)
            xr = xT.rearrange("(t p) s -> p t s", p=P)
            dma_engs = [nc.gpsimd, nc.sync, nc.scalar]
            for dt in range(DT):
                dma_engs[dt % 3].dma_start(xT_sb[:, dt, :], xr[:, dt, :])
            xq_sb = sing.tile([P, DT, SQ], BF16)
            xqr = xqT.rearrange("(t p) s -> p t s", p=P)
            for dt in range(DT):
                dma_engs[dt % 3].dma_start(xq_sb[:, dt, :], xqr[:, dt, :])
            mask_sb = sing.tile([P, KT, SQ], BF16)
            nc.gpsimd.dma_start(mask_sb[:], m01T.rearrange("(t p) q -> p t q", p=P))
            qT_sb = sing.tile([P, DT, SQ], BF16)
            kT_sb = sing.tile([P, DT, S], BF16)
            v_res = sing.tile([P, KT, H, HD + 1], BF16)
            attnT_sb = sing.tile([P, DT, SQ], BF16)
            bq_sb = sing.tile([P, DT], F32)
            nc.sync.dma_start(bq_sb[:], bq.rearrange("(t p) -> p t", p=P))
            bk_sb = sing.tile([P, DT], F32)
            nc.sync.dma_start(bk_sb[:], bk.rearrange("(t p) -> p t", p=P))
            bv_sb = sing.tile([P, D], F32)
            nc.sync.dma_start(bv_sb[:], bv[:].partition_broadcast(P))
            fcb_sb = sing.tile([P, D], F32)
            nc.sync.dma_start(fcb_sb[:], fcb[:].partition_broadcast(P))
            ones_f32 = sing.tile([P, H], F32)
            nc.vector.memset(ones_f32[:], 1.0)

            # PE warm-up: ~5us of dummy matmuls while the input DMAs land, so
            # the HAM clock gate reaches K=8/8 before the real work starts.
            warm_sb = sing.tile([P, 512], BF16)
            nc.vector.memset(warm_sb[:], 0.0)
            with tc.tile_pool(name="wps", bufs=1, space="PSUM") as wps:
                wp = wps.tile([P, 512], F32, name="wp", tag="wp")
                for it in range(24):
                    nc.tensor.matmul(
                        wp[:], warm_sb[:, 0:P], warm_sb[:],
                        start=(it == 0), stop=(it == 23),
                    )
            for kt in range(KT):
                nc.vector.tensor_copy(v_res[:, kt, :, HD], ones_f32[:])

            # ---- phase 1: QKV projections ----
            with (
                tc.tile_pool(name="wres", bufs=1) as wres,
                tc.tile_pool(name="ppsum", bufs=2, space="PSUM") as ppsum,
            ):
                # Resident weights, loaded in column blocks spread over two
                # DMA queues so the first projection matmuls start early.
                wk_sb = wres.tile([P, DT, D], BF16)
                wv_sb = wres.tile([P, DT, D], BF16)
                wq_sb = wres.tile([P, DT, D], BF16)
                wkr = wkT.rearrange("(t p) f -> p t f", p=P)
                wvr = wvT.rearrange("(t p) f -> p t f", p=P)
                wqr = wqT.rearrange("(t p) f -> p t f", p=P)
                engs = [nc.sync, nc.scalar]
                for ft in range(DT):
                    engs[ft % 2].dma_start(
                        wk_sb[:, :, ft * P:(ft + 1) * P], wkr[:, :, ft * P:(ft + 1) * P]
                    )
                for ft in range(DT):
                    engs[ft % 2].dma_start(
                        wq_sb[:, :, ft * P:(ft + 1) * P], wqr[:, :, ft * P:(ft + 1) * P]
                    )
                for ft in range(DT):
                    engs[ft % 2].dma_start(
                        wv_sb[:, :, ft * P:(ft + 1) * P], wvr[:, :, ft * P:(ft + 1) * P]
                    )

                # K_T for the full batch: [f, s]
                for ft in range(DT):
                    ps4 = ppsum.tile([P, SC, 512], F32, name="ps4", tag="ps4")
                    for sc in range(S // 512):
                        for dt in range(DT):
                            nc.tensor.matmul(
                                ps4[:, sc, :],
                                wk_sb[:, dt, ft * P:(ft + 1) * P],
                                xT_sb[:, dt, sc * 512:(sc + 1) * 512],
                                start=(dt == 0), stop=(dt == DT - 1),
                            )
                    nc.vector.tensor_scalar(
                        out=kT_sb[:, ft, :],
                        in0=ps4[:, 0:S // 512, :].rearrange("p c s -> p (c s)"),
                        scalar1=bk_sb[:, ft:ft + 1], scalar2=None,
                        op0=mybir.AluOpType.add,
                    )

                # Q_T for own rows only: [f, q]
                for ft in range(DT):
                    ps4 = ppsum.tile([P, SC, 512], F32, name="ps4", tag="ps4")
                    for sc in range(QSC):
                        for dt in range(DT):
                            nc.tensor.matmul(
                                ps4[:, sc, 0:QW],
                                wq_sb[:, dt, ft * P:(ft + 1) * P],
                                xq_sb[:, dt, sc * QW:(sc + 1) * QW],
                                start=(dt == 0), stop=(dt == DT - 1),
                            )
                    nc.vector.tensor_scalar(
                        out=qT_sb[:, ft, :],
                        in0=ps4[:, 0:QSC, 0:QW].rearrange("p c s -> p (c s)"),
                        scalar1=bq_sb[:, ft:ft + 1], scalar2=None,
                        op0=mybir.AluOpType.add,
                    )

                # V for the full batch, laid out [s, head, 65] with ones in
                # column 64 (the PV matmul's 65th output row accumulates the
                # softmax denominator for free).
                for st in range(KT):
                    ps4 = ppsum.tile([P, SC, 512], F32, name="ps4", tag="ps4")
                    for half in range(2):
                        for dt in range(DT):
                            nc.tensor.matmul(
                                ps4[:, half, :],
                                xT_sb[:, dt, st * P:(st + 1) * P],
                                wv_sb[:, dt, half * 512:(half + 1) * 512],
                                start=(dt == 0), stop=(dt == DT - 1),
                            )
                    nc.vector.tensor_tensor(
                        v_res[:, st, :, 0:HD],
                        ps4[:, 0:2, :].rearrange("p c s -> p (c s)")
                        .rearrange("p (h d) -> p h d", h=H),
                        bv_sb[:].rearrange("p (h d) -> p h d", h=H),
                        mybir.AluOpType.add,
                    )

            # fc weights: needed only in phase 3; loaded during attention in
            # a pool that begins after the projection-weight pool has closed.
            late_cm = tc.tile_pool(name="late", bufs=1)
            late = late_cm.__enter__()
            fcw_sb = late.tile([P, DT, D], BF16)
            nc.gpsimd.dma_start(fcw_sb[:], fcwT.rearrange("(t p) e -> p t e", p=P))

            # ---- phase 2: attention ----
            # Software-pipelined by one kt: the QK matmuls for kt+1 are
            # emitted (and sit in the PE FIFO) before exp/mask/PV of kt, so
            # the ScalarE exp stream (the pacing engine) stays dense.
            with (
                tc.tile_pool(name="probs", bufs=6) as probs_pool,
                tc.tile_pool(name="rnorm", bufs=2) as rnorm,
                tc.tile_pool(name="spsum", bufs=2, space="PSUM") as spsum,
                tc.tile_pool(name="apsum", bufs=2, space="PSUM") as apsum,
            ):
                for g in range(GROUPS):
                    attn_psA = apsum.tile([HD + 1, SQ], F32, name="attn_psA", tag="attn_psA")
                    attn_psB = apsum.tile([HD + 1, SQ], F32, name="attn_psB", tag="attn_psB")
                    attn_ps = [attn_psA, attn_psB]

                    def emit_qk(kt):
                        sc_ps = spsum.tile([P, 2, SQ], F32, name="sc_ps", tag="sc_ps")
                        for i in range(2):
                            nc.tensor.matmul(
                                sc_ps[:, i, :],
                                kT_sb[64 * i:64 * i + 64, g, kt * P:(kt + 1) * P],
                                qT_sb[64 * i:64 * i + 64, g, :],
                                start=True, stop=True,
                            )
                        return sc_ps

                    def emit_back(kt, sc_ps):
                        pr = probs_pool.tile([P, 2, SQ], BF16, name="pr", tag="pr")
                        nc.scalar.activation(
                            pr[:], sc_ps[:],
                            mybir.ActivationFunctionType.Exp,
                            scale=1.0 / math.sqrt(HD),
                        )
                        nc.vector.tensor_tensor(
                            pr[:], pr[:],
                            mask_sb[:, kt:kt + 1, :].to_broadcast((P, 2, SQ)),
                            mybir.AluOpType.mult,
                        )
                        for i in range(2):
                            nc.tensor.matmul(
                                attn_ps[i][:],
                                v_res[:, kt, 2 * g + i, :],
                                pr[:, i, :],
                                start=(kt == 0), stop=(kt == KT - 1),
                            )

                    pend = None
                    for kt in range(KT):
                        sc = emit_qk(kt)
                        if pend is not None:
                            emit_back(kt - 1, pend)
                        pend = sc
                    emit_back(KT - 1, pend)

                    # normalize: attn_T[dv, q] *= recip(sums[q]); row 64 of
                    # each accumulator is the softmax denominator.
                    rsA = rnorm.tile([1, SQ], F32, name="rsA", tag="rsA")
                    rsB = rnorm.tile([1, SQ], F32, name="rsB", tag="rsB")
                    nc.vector.tensor_copy(rsA[:], attn_psA[HD:HD + 1, :])
                    nc.vector.tensor_copy(rsB[:], attn_psB[HD:HD + 1, :])
                    rbA = rnorm.tile([HD, SQ], F32, name="rbA", tag="rbA")
                    rbB = rnorm.tile([HD, SQ], F32, name="rbB", tag="rbB")
                    nc.gpsimd.partition_broadcast(rbA[:], rsA[:])
                    nc.gpsimd.partition_broadcast(rbB[:], rsB[:])
                    nc.vector.reciprocal_approx_fast(rbA[:], rbA[:])
                    nc.vector.reciprocal_approx_fast(rbB[:], rbB[:])
                    nc.vector.tensor_tensor(
                        attnT_sb[0:HD, g, :], attn_psA[0:HD, :], rbA[:],
                        mybir.AluOpType.mult,
                    )
                    # head B lands on partitions 64-127 of attnT_sb; DVE can't
                    # shift partitions, so normalize at base 0 then DMA-shift.
                    tmpB = rnorm.tile([HD, SQ], BF16, name="tmpB", tag="tmpB")
                    nc.vector.tensor_tensor(
                        tmpB[:], attn_psB[0:HD, :], rbB[:], mybir.AluOpType.mult
                    )
                    nc.sync.dma_start(attnT_sb[HD:2 * HD, g, :], tmpB[:])

            # ---- phase 3: output projection ----
            with (
                tc.tile_pool(name="fout", bufs=2) as fout,
                tc.tile_pool(name="fpsum", bufs=4, space="PSUM") as fpsum,
            ):
                for qc in range(QC):
                    ot = fout.tile([P, D], F32, name="ot", tag="ot")
                    for eh in range(2):
                        ps = fpsum.tile([P, 512], F32, name="fps", tag="fps")
                        for dt in range(DT):
                            nc.tensor.matmul(
                                ps[:],
                                attnT_sb[:, dt, qc * P:(qc + 1) * P],
                                fcw_sb[:, dt, eh * 512:(eh + 1) * 512],
                                start=(dt == 0), stop=(dt == DT - 1),
                            )
                        nc.vector.tensor_tensor(
                            ot[:, eh * 512:(eh + 1) * 512], ps[:],
                            fcb_sb[:, eh * 512:(eh + 1) * 512],
                            mybir.AluOpType.add,
                        )
                    nc.sync.dma_start(out[qc * P:(qc + 1) * P, :], ot[:])

            late_cm.__exit__(None, None, None)

    nc.finalize()
    return nc


def make_pe(n, d):
    pos = np.arange(n, dtype=np.float32)[:, None]
    div = np.exp(
        np.arange(0, d, 2, dtype=np.float32) * (-math.log(10000.0) / d)
    ).astype(np.float32)
    pe = np.zeros((n, d), dtype=np.float32)
    pe[:, 0::2] = np.sin(pos * div)
    pe[:, 1::2] = np.cos(pos * div)
    return pe


def prep_in_maps(x, mask, qkv_w, qkv_b, fc_w, fc_b):
    x = np.asarray(x, dtype=np.float32)
    mask = np.asarray(mask)
    qkv_w = np.asarray(qkv_w, dtype=np.float32)
    qkv_b = np.asarray(qkv_b, dtype=np.float32)
    fc_w = np.asarray(fc_w, dtype=np.float32)
    fc_b = np.asarray(fc_b, dtype=np.float32)
    S = x.shape[1]
    SQ = S // 4
    pe = make_pe(B, D)
    bf16 = ml_dtypes.bfloat16
    wqT = np.ascontiguousarray(qkv_w[0:D].T.astype(bf16))
    wkT = np.ascontiguousarray(qkv_w[D:2 * D].T.astype(bf16))
    wvT = np.ascontiguousarray(qkv_w[2 * D:3 * D].T.astype(bf16))
    fcwT = np.ascontiguousarray(fc_w.T.astype(bf16))
    bv = np.ascontiguousarray(qkv_b[2 * D:3 * D])
    fcb = np.ascontiguousarray(fc_b)
    m01 = (mask[0, 0] != 1).astype(bf16)  # [S, S] keep-mask (q, k)
    xTb = [np.ascontiguousarray(x[b].T.astype(bf16)) for b in range(B)]
    in_maps = []
    for c in range(NCORES):
        b = c // 4
        q0 = (c % 4) * SQ
        in_maps.append({
            "xT": xTb[b],
            "xqT": np.ascontiguousarray(xTb[b][:, q0:q0 + SQ]),
            "wqT": wqT, "wkT": wkT, "wvT": wvT, "fcwT": fcwT,
            "bq": qkv_b[0:D] + pe[b],
            "bk": qkv_b[D:2 * D] + pe[b],
            "bv": bv, "fcb": fcb,
            "m01T": np.ascontiguousarray(m01[q0:q0 + SQ, :].T),
        })
    return in_maps


_NC_CACHE = {}


def run(x, mask, qkv_w, qkv_b, fc_w, fc_b, **spmd_kwargs):
    S = x.shape[1]
    if S not in _NC_CACHE:
        _NC_CACHE[S] = build_nc(S)
    nc = _NC_CACHE[S]
    in_maps = prep_in_maps(x, mask, qkv_w, qkv_b, fc_w, fc_b)
    res = run_bass_kernel_spmd(nc, in_maps, core_ids=list(range(NCORES)), **spmd_kwargs)
    SQ = S // 4
    full = np.empty((B, S, D), dtype=np.float32)
    for c in range(NCORES):
        b = c // 4
        q0 = (c % 4) * SQ
        full[b, q0:q0 + SQ, :] = res.results[c]["out"]
    return full, res


def kernel(x, mask, qkv_w, qkv_b, fc_w, fc_b):
    full, _ = run(x, mask, qkv_w, qkv_b, fc_w, fc_b)
    return full


# revision 54
# speedup vs baseline: 1.2827x; 1.2827x over previous
"""Trainium2 Bass kernel for nn_AttentionModule (B=2, S=2048, D=1024, H=16).

Sharding: 8 cores = 2 batches x 4 query blocks. Core c handles batch b=c//4,
query rows q0=(c%4)*SQ .. +SQ (SQ=512). Each core computes K and V for the
FULL batch locally (replicated within the 4-core batch group - cheaper and
far more predictable than exchanging K/V shards through the 4-rank AllGather,
whose latency measures ~140us on this part), plus Q for its own rows, then
runs attention for its query block and the output projection. Host-side
assembly is a pure concat.

Device math (all matmuls bf16 inputs, fp32 accumulation):
  K_T [D, S]     = Wk^T-tiles @ x_T    (+bias+pe via DVE per-partition add)
  V   [S, H, 65] = x_T-tiles @ Wv^T    (+bias; col 64 holds ones)
  Q_T [D, SQ]    = Wq^T-tiles @ xq_T   (own rows)
  S_T[k,q] = K_T^t @ Q_T per head (row-packed pairs, d=64)
  P = exp(S/8) on ScalarE (scale fused), masked by 0/1 bf16 multiply on DVE
  attn_T[dv,q] = V_aug^t @ P per head; row 64 accumulates the softmax sums
  attn = attn_T * recip(sums)  -> fc: out[q,e] = attn_T-tiles @ fcw^T-tiles

The attention loop is software-pipelined by one kt so the PE FIFO always has
the next QK pair ahead of the current PV, keeping the ScalarE exp stream (the
pacing engine) dense.
"""

import math

import ml_dtypes
import numpy as np

import concourse.bass as bass
import concourse.mybir as mybir
import concourse.tile as tile
from concourse import bacc
from concourse.bass_utils import run_bass_kernel_spmd

B = 2
D = 1024
H = 16
HD = 64
P = 128
NCORES = 8
F32 = mybir.dt.float32
BF16 = mybir.dt.bfloat16
FP8 = mybir.dt.float8e4


def build_nc(S=2048):
    SQ = S // 4          # query rows per core
    DT = D // P          # 8 d-model tiles
    KT = S // P          # key tiles
    QC = SQ // P         # query chunks for fc
    SC = max(S // 512, 2)  # 512-wide chunks in projection psum (>=2 for V)
    QSC = SQ // 512 if SQ >= 512 else 1
    QW = min(SQ, 512)    # q projection chunk width
    GROUPS = H // 2      # head pairs

    nc = bacc.Bacc(None)

    xT = nc.dram_tensor("xT", [D, S], BF16, kind="ExternalInput")
    wqT = nc.dram_tensor("wqT", [D, D], BF16, kind="ExternalInput")
    wkT = nc.dram_tensor("wkT", [D, D], BF16, kind="ExternalInput")
    wvT = nc.dram_tensor("wvT", [D, D], BF16, kind="ExternalInput")
    fcwT = nc.dram_tensor("fcwT", [D, D], BF16, kind="ExternalInput")
    bq = nc.dram_tensor("bq", [D], F32, kind="ExternalInput")
    bk = nc.dram_tensor("bk", [D], F32, kind="ExternalInput")
    bv = nc.dram_tensor("bv", [D], BF16, kind="ExternalInput")
    fcb = nc.dram_tensor("fcb", [D], BF16, kind="ExternalInput")
    m01T = nc.dram_tensor("m01T", [S, SQ], BF16, kind="ExternalInput")
    out = nc.dram_tensor("out", [SQ, D], F32, kind="ExternalOutput")

    with tile.TileContext(nc) as tc:
        with tc.tile_pool(name="sing", bufs=1) as sing:
            # ---- resident SBUF tensors ----
            xT_sb = sing.tile([P, DT, S], BF16)
            xr = xT.rearrange("(t p) s -> p t s", p=P)
            x_engs = [nc.gpsimd, nc.sync, nc.scalar, nc.gpsimd,
                      nc.sync, nc.scalar, nc.gpsimd, nc.sync]
            for dt in range(DT):
                x_engs[dt].dma_start(xT_sb[:, dt, :], xr[:, dt, :])
            mask_sb = sing.tile([P, KT, SQ], BF16)
            qT_sb = sing.tile([P, DT, SQ], BF16)
            kT_sb = sing.tile([P, DT, S], BF16)
            v_res = sing.tile([P, KT, H, HD + 1], BF16)
            attnT_sb = sing.tile([P, DT, SQ], BF16)
            bq_sb = sing.tile([P, DT], F32)
            nc.sync.dma_start(bq_sb[:], bq.rearrange("(t p) -> p t", p=P))
            bk_sb = sing.tile([P, DT], F32)
            nc.sync.dma_start(bk_sb[:], bk.rearrange("(t p) -> p t", p=P))
            bv_sb = sing.tile([P, D], BF16)
            nc.sync.dma_start(bv_sb[:], bv[:].partition_broadcast(P))
            fcb_sb = sing.tile([P, D], BF16)
            nc.sync.dma_start(fcb_sb[:], fcb[:].partition_broadcast(P))
            fcw_sb = sing.tile([P, DT, D], BF16)
            ones_f32 = sing.tile([P, H], F32)
            nc.vector.memset(ones_f32[:], 1.0)

            # PE warm-up: ~7us of dummy matmuls while the input DMAs land, so
            # the HAM clock gate reaches K=8/8 before the real work starts.
            warm_sb = sing.tile([P, HD], BF16)
            nc.vector.memset(warm_sb[:], 0.0)
            with tc.tile_pool(name="wps", bufs=1, space="PSUM") as wps:
                wp = wps.tile([HD, HD], F32, name="wp", tag="wp")
                for it in range(180):
                    nc.tensor.matmul(
                        wp[:], warm_sb[:], warm_sb[:],
                        start=(it == 0), stop=(it == 179),
                    )
            for kt in range(KT):
                nc.vector.tensor_copy(v_res[:, kt, :, HD], ones_f32[:])

            # ---- phase 1+2: projections interleaved with attention ----
            # V (full batch) is projected first; K/Q for head-pair g+1 are
            # dripped a few matmuls per kt into group g's attention stream
            # (via generators) so the PE never takes a bulk detour that would
            # starve the ScalarE exp pipeline. Projection psum chains share
            # the scores pool (same shape/tag): 3 slots x 2 banks + 2
            # accumulator banks = 8 PSUM banks.
            wres_cm = tc.tile_pool(name="wres", bufs=1)
            wres = wres_cm.__enter__()
            wk_sb = wres.tile([P, DT, D], BF16)
            wq_sb = wres.tile([P, DT, D], BF16)
            wkr = wkT.rearrange("(t p) f -> p t f", p=P)
            wvr = wvT.rearrange("(t p) f -> p t f", p=P)
            wqr = wqT.rearrange("(t p) f -> p t f", p=P)
            engs = [nc.sync, nc.scalar]
            for ft in range(DT):
                engs[ft % 2].dma_start(
                    wk_sb[:, :, ft * P:(ft + 1) * P], wkr[:, :, ft * P:(ft + 1) * P]
                )
            for ft in range(DT):
                engs[ft % 2].dma_start(
                    wq_sb[:, :, ft * P:(ft + 1) * P], wqr[:, :, ft * P:(ft + 1) * P]
                )

            KW = min(S, 1024)  # K projection pass width

            wv_cm = tc.tile_pool(name="wvpool", bufs=1)
            wvp = wv_cm.__enter__()
            wv_sb = wvp.tile([P, DT, D], BF16)
            wv_engs = [nc.gpsimd, nc.gpsimd, nc.sync, nc.scalar]
            for qtr in range(4):
                wv_engs[qtr].dma_start(
                    wv_sb[:, :, qtr * 256:(qtr + 1) * 256],
                    wvr[:, :, qtr * 256:(qtr + 1) * 256],
                )
            # late-needed loads go last in program order (queue priority)
            nc.gpsimd.dma_start(mask_sb[:], m01T.rearrange("(t p) q -> p t q", p=P))
            nc.gpsimd.dma_start(fcw_sb[:], fcwT.rearrange("(t p) e -> p t e", p=P))

            with (
                tc.tile_pool(name="spsum", bufs=3, space="PSUM") as spsum,
                tc.tile_pool(name="apsum", bufs=1, space="PSUM") as apsum,
            ):
                # V for the full batch, laid out [s, head, 65]; column 64
                # holds ones so PV's 65th row accumulates softmax sums.
                for st in range(KT):
                    ps2 = spsum.tile([P, 2, 512], F32, name="ps2", tag="sc_ps")
                    for half in range(2):
                        for dt in range(DT):
                            nc.tensor.matmul(
                                ps2[:, half, :],
                                xT_sb[:, dt, st * P:(st + 1) * P],
                                wv_sb[:, dt, half * 512:(half + 1) * 512],
                                start=(dt == 0), stop=(dt == DT - 1),
                            )
                    nc.vector.tensor_tensor(
                        v_res[:, st, :, 0:HD],
                        ps2[:].rearrange("p c s -> p (c s)")
                        .rearrange("p (h d) -> p h d", h=H),
                        bv_sb[:].rearrange("p (h d) -> p h d", h=H),
                        mybir.AluOpType.add,
                    )

                wv_cm.__exit__(None, None, None)
                probs_cm = tc.tile_pool(name="probs", bufs=8)
                probs_pool = probs_cm.__enter__()
                rnorm_cm = tc.tile_pool(name="rnorm", bufs=1)
                rnorm = rnorm_cm.__enter__()

                def gen_proj_kq(ft):
                    # K_T[:, ft, :] then Q_T[:, ft, :]; yields after each MM.
                    for pair in range(S // KW):
                        ps2 = spsum.tile([P, 2, 512], F32, name="ps2", tag="sc_ps")
                        for half in range(KW // 512):
                            sc = pair * (KW // 512) + half
                            for dt in range(DT):
                                nc.tensor.matmul(
                                    ps2[:, half, :],
                                    wk_sb[:, dt, ft * P:(ft + 1) * P],
                                    xT_sb[:, dt, sc * 512:(sc + 1) * 512],
                                    start=(dt == 0), stop=(dt == DT - 1),
                                )
                                yield
                        nc.vector.tensor_scalar(
                            out=kT_sb[:, ft, pair * KW:(pair + 1) * KW],
                            in0=ps2[:, 0:KW // 512, :].rearrange("p c s -> p (c s)"),
                            scalar1=bk_sb[:, ft:ft + 1], scalar2=None,
                            op0=mybir.AluOpType.add,
                        )
                    ps2 = spsum.tile([P, 2, 512], F32, name="ps2", tag="sc_ps")
                    for sc in range(QSC):
                        for dt in range(DT):
                            nc.tensor.matmul(
                                ps2[:, sc, 0:QW],
                                wq_sb[:, dt, ft * P:(ft + 1) * P],
                                xT_sb[:, dt, sc * QW:(sc + 1) * QW],
                                start=(dt == 0), stop=(dt == DT - 1),
                            )
                            yield
                    nc.vector.tensor_scalar(
                        out=qT_sb[:, ft, :],
                        in0=ps2[:, 0:QSC, 0:QW].rearrange("p c s -> p (c s)"),
                        scalar1=bq_sb[:, ft:ft + 1], scalar2=None,
                        op0=mybir.AluOpType.add,
                    )

                # first group's K/Q up-front
                for _ in gen_proj_kq(0):
                    pass

                for g in range(GROUPS):
                    attn_psA = apsum.tile([HD + 1, SQ], F32, name="attn_psA", tag="attn_psA")
                    attn_psB = apsum.tile([HD + 1, SQ], F32, name="attn_psB", tag="attn_psB")
                    attn_ps = [attn_psA, attn_psB]
                    nextgen = iter(gen_proj_kq(g + 1)) if g + 1 < GROUPS else None
                    # ~40 proj matmuls dripped over the first ~14 kts
                    drip = (40 + KT - 3) // max(KT - 2, 1)

                    def emit_qk(kt):
                        sc_ps = spsum.tile([P, 2, SQ], F32, name="sc_ps", tag="sc_ps")
                        for i in range(2):
                            nc.tensor.matmul(
                                sc_ps[:, i, :],
                                kT_sb[64 * i:64 * i + 64, g, kt * P:(kt + 1) * P],
                                qT_sb[64 * i:64 * i + 64, g, :],
                                start=True, stop=True,
                            )
                        return sc_ps

                    def emit_back(kt, sc_ps):
                        pr = probs_pool.tile([P, 2, SQ], BF16, name="pr", tag="pr")
                        nc.scalar.activation(
                            pr[:], sc_ps[:],
                            mybir.ActivationFunctionType.Exp,
                            scale=1.0 / math.sqrt(HD),
                        )
                        nc.vector.tensor_tensor(
                            pr[:], pr[:],
                            mask_sb[:, kt:kt + 1, :].to_broadcast((P, 2, SQ)),
                            mybir.AluOpType.mult,
                        )
                        for i in range(2):
                            nc.tensor.matmul(
                                attn_ps[i][:],
                                v_res[:, kt, 2 * g + i, :],
                                pr[:, i, :],
                                start=(kt == 0), stop=(kt == KT - 1),
                            )

                    pend = None
                    for kt in range(KT):
                        sc = emit_qk(kt)
                        if pend is not None:
                            emit_back(kt - 1, pend)
                        pend = sc
                        if nextgen is not None:
                            for _ in range(drip):
                                if next(nextgen, "end") == "end":
                                    nextgen = None
                                    break
                    emit_back(KT - 1, pend)
                    if nextgen is not None:
                        for _ in nextgen:
                            pass

                    # normalize: pull the accumulators out of PSUM with two
                    # quick copies (freeing the banks for the next group),
                    # then recip+multiply off the critical path.
                    # pull both accumulators out of PSUM with quick copies
                    # (freeing the banks for the next group's PVs), then
                    # recip+multiply off the critical path.
                    rs = rnorm.tile([1, SQ], F32, name="rs", tag="rs")
                    rsB = rnorm.tile([1, SQ], F32, name="rsB", tag="rsB")
                    cpA = rnorm.tile([HD, SQ], BF16, name="cpA", tag="cpA")
                    cpB = rnorm.tile([HD, SQ], BF16, name="cpB", tag="cpB")
                    nc.vector.tensor_copy(rs[:], attn_psA[HD:HD + 1, :])
                    nc.vector.tensor_copy(cpA[:], attn_psA[0:HD, :])
                    nc.vector.tensor_copy(rsB[:], attn_psB[HD:HD + 1, :])
                    nc.vector.tensor_copy(cpB[:], attn_psB[0:HD, :])
                    rb = rnorm.tile([HD, SQ], F32, name="rb", tag="rb")
                    nc.gpsimd.partition_broadcast(rb[:], rs[:])
                    nc.vector.reciprocal_approx_fast(rb[:], rb[:])
                    nc.vector.tensor_tensor(
                        attnT_sb[0:HD, g, :], cpA[:], rb[:], mybir.AluOpType.mult
                    )
                    # head B lands on partitions 64-127 of attnT_sb; DVE can't
                    # shift partitions, so normalize at base 0 then DMA-shift.
                    nc.gpsimd.partition_broadcast(rb[:], rsB[:])
                    nc.vector.reciprocal_approx_fast(rb[:], rb[:])
                    nc.vector.tensor_tensor(
                        cpB[:], cpB[:], rb[:], mybir.AluOpType.mult
                    )
                    nc.sync.dma_start(attnT_sb[HD:2 * HD, g, :], cpB[:])

                rnorm_cm.__exit__(None, None, None)
                probs_cm.__exit__(None, None, None)

            wres_cm.__exit__(None, None, None)

            # ---- phase 3: output projection ----
            with (
                tc.tile_pool(name="fout", bufs=2) as fout,
                tc.tile_pool(name="fpsum", bufs=4, space="PSUM") as fpsum,
            ):
                for qc in range(QC):
                    ot = fout.tile([P, D], F32, name="ot", tag="ot")
                    for eh in range(2):
                        ps = fpsum.tile([P, 512], F32, name="fps", tag="fps")
                        for dt in range(DT):
                            nc.tensor.matmul(
                                ps[:],
                                attnT_sb[:, dt, qc * P:(qc + 1) * P],
                                fcw_sb[:, dt, eh * 512:(eh + 1) * 512],
                                start=(dt == 0), stop=(dt == DT - 1),
                            )
                        nc.vector.tensor_tensor(
                            ot[:, eh * 512:(eh + 1) * 512], ps[:],
                            fcb_sb[:, eh * 512:(eh + 1) * 512],
                            mybir.AluOpType.add,
                        )
                    nc.sync.dma_start(out[qc * P:(qc + 1) * P, :], ot[:])

    nc.finalize()
    return nc


def make_pe(n, d):
    pos = np.arange(n, dtype=np.float32)[:, None]
    div = np.exp(
        np.arange(0, d, 2, dtype=np.float32) * (-math.log(10000.0) / d)
    ).astype(np.float32)
    pe = np.zeros((n, d), dtype=np.float32)
    pe[:, 0::2] = np.sin(pos * div)
    pe[:, 1::2] = np.cos(pos * div)
    return pe


def prep_in_maps(x, mask, qkv_w, qkv_b, fc_w, fc_b):
    x = np.asarray(x, dtype=np.float32)
    mask = np.asarray(mask)
    qkv_w = np.asarray(qkv_w, dtype=np.float32)
    qkv_b = np.asarray(qkv_b, dtype=np.float32)
    fc_w = np.asarray(fc_w, dtype=np.float32)
    fc_b = np.asarray(fc_b, dtype=np.float32)
    S = x.shape[1]
    SQ = S // 4
    pe = make_pe(B, D)
    bf16 = ml_dtypes.bfloat16
    wqT = np.ascontiguousarray(qkv_w[0:D].T.astype(bf16))
    wkT = np.ascontiguousarray(qkv_w[D:2 * D].T.astype(bf16))
    wvT = np.ascontiguousarray(qkv_w[2 * D:3 * D].T.astype(bf16))
    fcwT = np.ascontiguousarray(fc_w.T.astype(bf16))
    bv = np.ascontiguousarray(qkv_b[2 * D:3 * D])
    fcb = np.ascontiguousarray(fc_b)
    m01 = (mask[0, 0] != 1).astype(bf16)  # [S, S] keep-mask
    xTb = [np.ascontiguousarray(x[b].T.astype(bf16)) for b in range(B)]
    in_maps = []
    for c in range(NCORES):
        b = c // 4
        q0 = (c % 4) * SQ
        # Rotate the sequence axis so this core's query block sits first: the
        # kernel then reads Q inputs from xT[:, 0:SQ]. K/V inherit the rotated
        # order; attention is order-invariant over k as long as the mask's
        # k-axis is rotated identically.
        perm = np.r_[q0:S, 0:q0]
        in_maps.append({
            "xT": np.ascontiguousarray(xTb[b][:, perm]),
            "wqT": wqT, "wkT": wkT, "wvT": wvT, "fcwT": fcwT,
            "bq": qkv_b[0:D] + pe[b],
            "bk": qkv_b[D:2 * D] + pe[b],
            "bv": bv.astype(ml_dtypes.bfloat16), "fcb": fcb.astype(ml_dtypes.bfloat16),
            "m01T": np.ascontiguousarray(m01[q0:q0 + SQ, :][:, perm].T),
        })
    return in_maps


_NC_CACHE = {}


def run(x, mask, qkv_w, qkv_b, fc_w, fc_b, **spmd_kwargs):
    S = x.shape[1]
    if S not in _NC_CACHE:
        _NC_CACHE[S] = build_nc(S)
    nc = _NC_CACHE[S]
    in_maps = prep_in_maps(x, mask, qkv_w, qkv_b, fc_w, fc_b)
    res = run_bass_kernel_spmd(nc, in_maps, core_ids=list(range(NCORES)), **spmd_kwargs)
    SQ = S // 4
    full = np.empty((B, S, D), dtype=np.float32)
    for c in range(NCORES):
        b = c // 4
        q0 = (c % 4) * SQ
        full[b, q0:q0 + SQ, :] = res.results[c]["out"]
    return full, res


def kernel(x, mask, qkv_w, qkv_b, fc_w, fc_b):
    full, _ = run(x, mask, qkv_w, qkv_b, fc_w, fc_b)
    return full
